# revision 1
# baseline (speedup 1.0000x reference)
"""APoT (additive powers-of-two) fake-quant forward kernel for Trainium2.

y = sign(x) * Q(|x| / (alpha+eps)) * alpha, with Q the 2-bank greedy APoT
projection from the reference (BITS=5, K=2), applied elementwise to an
8192x8192 f32 tensor, sharded row-wise across 8 NeuronCores.

Math (exactly equivalent to the reference, verified on 6M+ points):
  t    = 32 * |x| / (alpha+eps)            (reference clips t<=32; see below)
  G    = staircase(t; thresholds {1,5,20} -> {0,1,4,16})       (bank-0, x4 geometric)
  w    = t - 2G
  G2   = staircase(w; thresholds {0.5,2.5,10} -> {0,1,4,16})   (bank-1)
  acc32= 2G + G2   in {0,1,2,3,6,8,9,12,24,32}
  y    = sign(x) * acc32 * alpha / (32*(1.5+1e-8))

Implementation works in T = 2t units so that all staircase constants fit the
custom-DVE const slots:
  ACT:  T = Abs(x * 64/(alpha+eps));  sg = Sign(x);  y = K2 * sacc
  DVE:  OP1 (custom): mq  = 4G   from T        (thresholds {2,10,40})
        OP2 (custom): g2d = 2*G2 from (T, mq)  (thresholds {1,5,20} on T-mq)
        acc2 = g2d + mq  (= 2*acc32, bf16-exact integers)
        sacc = min(acc2, 64) * sg      (min also realizes the reference's
                                        t<=32 clip: for T>64 only the top
                                        level 64 is legal)
  K2 = alpha / (64 * (1.5+1e-8))
"""

import os
import sys

import numpy as np

for _p in ("/opt/trn_rl_repo", "/root/.axon_site/_ro/trn_rl_repo"):
    if os.path.isdir(_p) and _p not in sys.path:
        sys.path.insert(0, _p)

import concourse.tile as tile
from concourse import bacc, mybir
from concourse.bass_utils import run_bass_kernel_spmd
from concourse.dve_ops import (
    CUSTOM_DVE_SPECS,
    OPS,
    _CUSTOM_DVE_ROW_BASE,
    _SUB_OPCODE_FOR_NAME,
    DveOp,
    has_src1,
)
from concourse.dve_spec import C0, C1, C2, One, Spec, Src0, Src1, lower, select, sq
from concourse.dve_uop import DveOpSpec

N_CORES = 8
EPS = 1e-8
LMAX_EPS = 1.5 + 1e-8


def _register(name: str, spec: Spec) -> DveOp:
    """Register a custom DVE op at runtime (append-only, idempotent)."""
    for op in OPS:
        if op.name == name:
            return op
    opcode = _CUSTOM_DVE_ROW_BASE + len(OPS)
    assert opcode < 0x20
    _SUB_OPCODE_FOR_NAME[name] = opcode
    sha = {}
    for ver in ("v3",):
        s = DveOpSpec(name=name, opcode=opcode, uops=lower(spec, ver=ver),
                      rd1_en=has_src1(spec))
        sha[ver] = s.sha(ver)
    op = DveOp(name, spec, subdim=False, uops_sha=sha)
    OPS.append(op)
    CUSTOM_DVE_SPECS[name] = spec
    return op


def _build_specs():
    # OP1: in0=T. out = mq = 4*G. G staircase thresholds on T: {2,10,40}.
    j0 = Src0 >= C0           # C0 = 2
    j1 = Src0 >= C1           # C1 = 10
    sB = j0 + j1              # {0,1,2}; thresholds nested so sum == select
    j2 = Src0 >= C2           # C2 = 40
    FOUR = C0 * C0            # stream-invariant -> latch, zero stages
    m = select(j2, FOUR, sB)  # m in {0,1,2,4}
    md = m + m
    op1 = _register("APOT_BANK0", Spec(body=sq(md)))  # (2m)^2 = 4G

    # OP2: in0=T, in1=mq. out = g2d = 2*G2 on W = T - mq (thresholds {1,5,20}).
    W = Src0 - Src1
    i0 = W >= One
    i1 = W >= C0              # C0 = 5
    sA = i0 + i1
    i2 = W >= C1              # C1 = 20
    mm = select(i2, C2, sA)   # C2 = 4 -> mm in {0,1,2,4}
    mmd = mm + mm
    op2 = _register("APOT_BANK1", Spec(body=mm * mmd))  # 2*mm^2 = 2*G2
    return op1, op2


def _build_nc(alpha: float, sh_rows: int, cols: int, fd: int = 2048):
    """Build + compile the per-core Bass graph for a [sh_rows, cols] f32 shard."""
    op1, op2 = _build_specs()
    scale_t = float(np.float32(64.0 / (np.float64(alpha) + EPS)))
    k2 = float(np.float32(np.float64(alpha) / (64.0 * LMAX_EPS)))

    nc = bacc.Bacc("TRN2", target_bir_lowering=False, debug=False,
                   num_devices=N_CORES)
    x_ap = nc.dram_tensor("x", [sh_rows, cols], mybir.dt.float32,
                          kind="ExternalInput").ap()
    out_ap = nc.dram_tensor("out", [sh_rows, cols], mybir.dt.float32,
                            kind="ExternalOutput").ap()

    f32, bf16 = mybir.dt.float32, mybir.dt.bfloat16
    Act = mybir.ActivationFunctionType
    Alu = mybir.AluOpType
    n_r, n_c = sh_rows // 128, cols // fd

    with tile.TileContext(nc) as tc:
        with tc.tile_pool(name="io", bufs=3) as iop, \
             tc.tile_pool(name="tmp", bufs=3) as tmp:
            for r in range(n_r):
                for c in range(n_c):
                    rs, cs = 128 * r, fd * c
                    xt = iop.tile([128, fd], f32, tag="x")
                    nc.sync.dma_start(xt[:], x_ap[rs:rs + 128, cs:cs + fd])

                    tt = tmp.tile([128, fd], f32, tag="T")
                    nc.scalar.activation(tt[:], xt[:], Act.Abs, scale=scale_t)
                    sg = tmp.tile([128, fd], bf16, tag="sg")
                    nc.scalar.activation(sg[:], xt[:], Act.Sign)

                    mq = tmp.tile([128, fd], bf16, tag="mq")
                    nc.vector._custom_dve(op1, out=mq[:], in0=tt[:],
                                          s0=2.0, s1=10.0, imm2=40.0)
                    g2d = tmp.tile([128, fd], bf16, tag="g2d")
                    nc.vector._custom_dve(op2, out=g2d[:], in0=tt[:], in1=mq[:],
                                          s0=5.0, s1=20.0, imm2=4.0)
                    acc = tmp.tile([128, fd], bf16, tag="acc")
                    nc.vector.tensor_add(acc[:], g2d[:], mq[:])
                    sacc = tmp.tile([128, fd], bf16, tag="sacc")
                    nc.vector.scalar_tensor_tensor(sacc[:], acc[:], 64.0, sg[:],
                                                   op0=Alu.min, op1=Alu.mult)

                    yt = iop.tile([128, fd], f32, tag="y")
                    nc.scalar.mul(yt[:], sacc[:], k2)
                    nc.sync.dma_start(out_ap[rs:rs + 128, cs:cs + fd], yt[:])
    nc.compile()
    return nc


_NC_CACHE: dict = {}


def _get_nc(alpha: float, sh_rows: int, cols: int):
    key = (round(float(alpha), 12), sh_rows, cols)
    if key not in _NC_CACHE:
        _NC_CACHE[key] = _build_nc(float(alpha), sh_rows, cols)
    return _NC_CACHE[key]


def run(x: np.ndarray, alpha: np.ndarray, trace: bool = False):
    """Shard, run on 8 cores, gather. Returns (y, BassKernelResults)."""
    x = np.ascontiguousarray(x, dtype=np.float32)
    rows, cols = x.shape
    assert rows % N_CORES == 0
    sh_rows = rows // N_CORES
    nc = _get_nc(float(alpha), sh_rows, cols)
    shards = np.split(x, N_CORES, axis=0)
    in_maps = [{"x": s} for s in shards]
    res = run_bass_kernel_spmd(nc, in_maps, core_ids=list(range(N_CORES)),
                               trace=trace)
    y = np.concatenate([res.results[i]["out"] for i in range(N_CORES)], axis=0)
    return y.astype(np.float32, copy=False), res


def kernel(x: np.ndarray, alpha: np.ndarray) -> np.ndarray:
    y, _ = run(x, alpha)
    return y
